# revision 26
# baseline (speedup 1.0000x reference)
"""Trainium2 Bass kernel for nn_Encoder_inter: coif1 wavelet disentangle along
the node axis (expressed as a dense 512x512 matrix, precomputed on host) followed
by a 2-layer MLP (64->256->256) with ReLU, pointwise over (B, N, T).

Sharding: data-parallel over batch B=32 across 8 NeuronCores (4 batches each);
the small Linear weights and the wavelet matrix are replicated.
"""
import os
import sys

for _p in ("/opt/trn_rl_repo", "/root/.axon_site/_ro/trn_rl_repo"):
    if os.path.isdir(_p) and _p not in sys.path:
        sys.path.insert(0, _p)

from contextlib import ExitStack

import numpy as np

import concourse.bass as bass
import concourse.tile as tile
from concourse import bacc, mybir
from concourse.bass_utils import run_bass_kernel_spmd

F32 = mybir.dt.float32
F32R = mybir.dt.float32r
BF16 = mybir.dt.bfloat16

# compute dtype for tensor-engine operands: "bf16" or "f32r"
COMPUTE = os.environ.get("KERNEL_COMPUTE_DTYPE", "bf16")
MM_DT = BF16 if COMPUTE == "bf16" else F32R

B, N, T, D, H, G = 32, 512, 24, 64, 256, 256
NCORES = 8
BPC = B // NCORES          # batches per core
TD = T * D                 # 1536
NCHUNK = N // 128          # 4
MCHUNK = N // 128          # 4
THALF = T // 2             # 12

# ---------------------------------------------------------------------------
# Host-side wavelet matrix: the whole dwt -> (2*cD) -> idwt chain along the
# node axis is linear, so it is exactly y = K @ x with K (N, N). We build
# K^T = op(eye(N)) in float64 with a numpy port of the reference transform.
# ---------------------------------------------------------------------------
_L = 6
_DEC_LO = np.array(
    [-0.01565572813546454, -0.0727326195128539, 0.38486484686420286,
     0.8525720202122554, 0.3378976624578092, -0.0727326195128539],
    dtype=np.float64,
)
_DEC_HI = np.array(
    [0.0727326195128539, 0.3378976624578092, -0.8525720202122554,
     0.38486484686420286, 0.0727326195128539, -0.01565572813546454],
    dtype=np.float64,
)
_REC_LO = _DEC_LO[::-1].copy()
_REC_HI = _DEC_HI[::-1].copy()


def _dwt_last(x):
    n = x.shape[-1]
    ext = np.concatenate(
        [x[..., : _L - 1][..., ::-1], x, x[..., -(_L - 1):][..., ::-1]], axis=-1
    )
    out = (n + _L - 2) // 2
    cA = sum(_DEC_LO[j] * ext[..., _L - j: _L - j + 2 * out: 2] for j in range(_L))
    cD = sum(_DEC_HI[j] * ext[..., _L - j: _L - j + 2 * out: 2] for j in range(_L))
    return cA, cD


def _idwt_last(cA, cD, n):
    out = cA.shape[-1]
    up_shape = cA.shape[:-1] + (2 * out - 1,)
    upA = np.zeros(up_shape, cA.dtype)
    upA[..., ::2] = cA
    upD = np.zeros(up_shape, cD.dtype)
    upD[..., ::2] = cD
    pad = [(0, 0)] * (cA.ndim - 1) + [(_L - 1, _L - 1)]
    uA = np.pad(upA, pad)
    uD = np.pad(upD, pad)
    return sum(
        _REC_LO[j] * uA[..., 2 * _L - 3 - j: 2 * _L - 3 - j + n]
        + _REC_HI[j] * uD[..., 2 * _L - 3 - j: 2 * _L - 3 - j + n]
        for j in range(_L)
    )


def _wavelet_kt() -> np.ndarray:
    """K^T (m_in, n_out) so that (op(x))[n] = sum_m x[m] * KT[m, n]."""
    eye = np.eye(N, dtype=np.float64)
    cA, cD = _dwt_last(eye)
    kt = _idwt_last(cA, 2.0 * cD, N)
    return kt.astype(np.float32)


# ---------------------------------------------------------------------------
# Device kernel (SPMD, identical program on all 8 cores)
# ---------------------------------------------------------------------------
_NC_CACHE = None


def _build_nc():
    nc = bacc.Bacc("TRN2", target_bir_lowering=False, debug=False, num_devices=NCORES)
    x_d = nc.dram_tensor("x", [BPC, MCHUNK, 128, TD], MM_DT, kind="ExternalInput").ap()
    kt_d = nc.dram_tensor("KT", [MCHUNK, 128, N], MM_DT, kind="ExternalInput").ap()
    w1_d = nc.dram_tensor("W1T", [2 * D, H], MM_DT, kind="ExternalInput").ap()
    w2_d = nc.dram_tensor("W2T", [2, 128, G], MM_DT, kind="ExternalInput").ap()
    s1_d = nc.dram_tensor("s1", [2, 128, 1], F32, kind="ExternalInput").ap()
    s2_d = nc.dram_tensor("s2", [2, 128, 1], F32, kind="ExternalInput").ap()
    out_d = nc.dram_tensor("out", [BPC, N, T, G], MM_DT, kind="ExternalOutput").ap()

    relu = mybir.ActivationFunctionType.Relu

    with tile.TileContext(nc) as tc, ExitStack() as ctx:
        consts = ctx.enter_context(tc.tile_pool(name="consts", bufs=1))
        xpool = ctx.enter_context(tc.tile_pool(name="xp", bufs=3))
        ypool = ctx.enter_context(tc.tile_pool(name="yp", bufs=4))
        hpool = ctx.enter_context(tc.tile_pool(name="hp", bufs=3))
        spool = ctx.enter_context(tc.tile_pool(name="sp", bufs=3))
        py = ctx.enter_context(tc.tile_pool(name="py", bufs=2, space="PSUM"))
        ph = ctx.enter_context(tc.tile_pool(name="ph", bufs=2, space="PSUM"))
        po = ctx.enter_context(tc.tile_pool(name="po", bufs=2, space="PSUM"))

        # --- replicated constants ---
        kt_sb = []
        for mc in range(MCHUNK):
            t_ = consts.tile([128, N], MM_DT, tag=f"kt{mc}", name=f"kt{mc}")
            nc.gpsimd.dma_start(out=t_[:], in_=kt_d[mc])
            kt_sb.append(t_)
        w1_sb = consts.tile([2 * D, H], MM_DT, tag="w1", name="w1")
        nc.gpsimd.dma_start(out=w1_sb[:], in_=w1_d[:])
        w2_sb = []
        for hc in range(2):
            t_ = consts.tile([128, G], MM_DT, tag=f"w2{hc}", name=f"w2{hc}")
            nc.gpsimd.dma_start(out=t_[:], in_=w2_d[hc])
            w2_sb.append(t_)
        s1_sb, s2_sb = [], []
        for hc in range(2):
            t_ = consts.tile([128, 1], F32, tag=f"s1{hc}", name=f"s1c{hc}")
            nc.gpsimd.dma_start(out=t_[:], in_=s1_d[hc])
            s1_sb.append(t_)
            t_ = consts.tile([128, 1], F32, tag=f"s2{hc}", name=f"s2c{hc}")
            nc.gpsimd.dma_start(out=t_[:], in_=s2_d[hc])
            s2_sb.append(t_)

        for b in range(BPC):
            x_sb = [
                xpool.tile([128, TD], MM_DT, tag=f"x{mc}", name=f"xt{mc}")
                for mc in range(MCHUNK)
            ]
            for hf in range(2):
                for mc in range(MCHUNK):
                    eng = nc.sync if mc % 2 == 0 else nc.scalar
                    eng.dma_start(
                        out=x_sb[mc][:, hf * (TD // 2):(hf + 1) * (TD // 2)],
                        in_=x_d[b, mc][:, hf * (TD // 2):(hf + 1) * (TD // 2)],
                    )
            for half in range(2):
                stgs = [
                    spool.tile(
                        [128, NCHUNK * (THALF // 2) * G], MM_DT, tag=f"stg{q}",
                        name=f"stg{q}",
                    )
                    for q in range(2)
                ]
                stg4s = [
                    s[:].rearrange("p (k t g) -> p k t g", k=NCHUNK, t=THALF // 2)
                    for s in stgs
                ]
                for tp in range(THALF // 2):
                    t0 = half * THALF + 2 * tp
                    # step 1 (t-pair): psum rows = [t0 d | t1 d], cols = n
                    yps = py.tile([128, N], F32, name="yps")
                    for mc in range(MCHUNK):
                        if mc == 0:
                            windows = [(0, 132, True)]
                        else:
                            windows = [
                                (128 * mc - 4, min(N, 128 * mc + 132), False)
                            ]
                        for lo, hi, st in windows:
                            nc.tensor.matmul(
                                yps[:, lo:hi],
                                lhsT=x_sb[mc][:, t0 * D:(t0 + 2) * D],
                                rhs=kt_sb[mc][:, lo:hi],
                                start=st,
                                stop=(mc == MCHUNK - 1),
                                skip_group_check=True,
                            )
                    y_sb = ypool.tile([128, N], MM_DT, tag="yt", name="y_sb")
                    if tp % 2 == 0:
                        nc.scalar.copy(y_sb[:], yps[:])
                    else:
                        nc.vector.tensor_copy(y_sb[:], yps[:])
                    # step 2: per hc, both t of the pair into one 2-bank psum
                    h1 = []
                    for hc in range(2):
                        hps = ph.tile([128, 2 * N], F32, name="hps")
                        for qq in range(2):
                            for ti in range(2):
                                nc.tensor.matmul(
                                    hps[:, ti * N + qq * 256:ti * N + (qq + 1) * 256],
                                    lhsT=w1_sb[
                                        ti * D:(ti + 1) * D, hc * 128:(hc + 1) * 128
                                    ],
                                    rhs=y_sb[ti * D:(ti + 1) * D, qq * 256:(qq + 1) * 256],
                                    start=(qq == 0),
                                    stop=(qq == 1),
                                    skip_group_check=True,
                                    tile_position=(ti * D, 0),
                                )
                        h_sb = hpool.tile(
                            [128, 2 * N], MM_DT, tag=f"h1_{hc}", name=f"h1_{hc}"
                        )
                        nc.vector.tensor_scalar(
                            h_sb[:], hps[:], s1_sb[hc][:], s2_sb[hc][:],
                            mybir.AluOpType.add, mybir.AluOpType.max,
                        )
                        h1.append(h_sb)
                    # step 3: nck-pairs share one psum bank (128, 512)
                    for ti in range(2):
                        tl = 2 * tp + ti
                        for nckp in range(NCHUNK // 2):
                            ops = po.tile([128, 2 * G], F32, name="ops")
                            for sub in range(2):
                                nck = 2 * nckp + sub
                                for hc in range(2):
                                    nc.tensor.matmul(
                                        ops[:, sub * G:(sub + 1) * G],
                                        lhsT=h1[hc][
                                            :,
                                            ti * N + nck * 128:ti * N + (nck + 1) * 128,
                                        ],
                                        rhs=w2_sb[hc][:],
                                        start=(sub == 0 and hc == 0),
                                        stop=(sub == 1 and hc == 1),
                                        skip_group_check=True,
                                    )
                            nc.scalar.activation(
                                stg4s[tl // 6][:, 2 * nckp:2 * nckp + 2, tl % 6, :],
                                ops[:].rearrange("p (k g) -> p k g", k=2),
                                relu,
                            )
                for q in range(2):
                    tq = THALF // 2
                    for nck in range(NCHUNK):
                        oeng = nc.sync if nck % 2 == 0 else nc.scalar
                        oeng.dma_start(
                            out=out_d[
                                b,
                                nck * 128:(nck + 1) * 128,
                                half * THALF + q * tq:half * THALF + (q + 1) * tq,
                                :,
                            ],
                            in_=stgs[q][
                                :, nck * tq * G:(nck + 1) * tq * G
                            ].rearrange("p (t g) -> p t g", t=tq),
                        )
    nc.compile()
    return nc


def _get_nc():
    global _NC_CACHE
    if _NC_CACHE is None:
        _NC_CACHE = _build_nc()
    return _NC_CACHE


def _make_in_maps(x, W1, b1, W2, b2):
    if COMPUTE == "bf16":
        import ml_dtypes
        mmnp = ml_dtypes.bfloat16
    else:
        mmnp = np.float32
    x = np.ascontiguousarray(np.asarray(x, dtype=np.float32))
    W1 = np.asarray(W1, dtype=np.float32)
    b1 = np.asarray(b1, dtype=np.float32)
    W2 = np.asarray(W2, dtype=np.float32)
    b2 = np.asarray(b2, dtype=np.float32)

    kt = _wavelet_kt().reshape(MCHUNK, 128, N).astype(mmnp)
    w1t = np.ascontiguousarray(np.concatenate([W1.T, W1.T], axis=0)).astype(mmnp)
    w2t = np.ascontiguousarray(W2.T).reshape(2, 128, G).astype(mmnp)
    c = np.linalg.solve(W2.astype(np.float64), b2.astype(np.float64))
    s1 = np.ascontiguousarray((b1.astype(np.float64) + c).astype(np.float32)
                              .reshape(2, 128, 1))
    s2 = np.ascontiguousarray(c.astype(np.float32).reshape(2, 128, 1))

    in_maps = []
    for c in range(NCORES):
        xc = x[c * BPC:(c + 1) * BPC].reshape(BPC, N, TD)
        xc = np.ascontiguousarray(xc.reshape(BPC, MCHUNK, 128, TD).astype(mmnp))
        in_maps.append(
            {"x": xc, "KT": kt, "W1T": w1t, "W2T": w2t, "s1": s1, "s2": s2}
        )
    return in_maps


def kernel(x, W1, b1, W2, b2):
    nc = _get_nc()
    in_maps = _make_in_maps(x, W1, b1, W2, b2)
    res = run_bass_kernel_spmd(nc, in_maps, list(range(NCORES)))
    out = np.concatenate([res.results[c]["out"] for c in range(NCORES)], axis=0)
    return np.ascontiguousarray(out.astype(np.float32))
